# revision 1
# baseline (speedup 1.0000x reference)
"""Trainium2 Bass kernel for nn_KeypointLoss (S=3, B=8, K=11, C=23, H=W=256).

Data-parallel over batch B=8 across 8 NeuronCores: core b computes the three
losses (heatmap / label / mask) for batch element b; host assembles [B,S].

Per-core device algorithm (all loss math on device), per stack s:
  heat : one batched DVE mul (mask broadcast over K), one DVE sub, one ACT
         Square with accum -> acc col s
  label: per-plane argmax = DVE row-max + PE transpose + one-hot; winning gt
         row re-fetched via one indirect DMA to recover the column; the 7
         label-channel values gathered with one width-1 indirect DMA; BCE on
         [11,7]
  mask : BCE via ACT Ln(+accum) and DVE fused mul-reduce
  final: two small matmuls reduce partition partials -> out[1,9]
"""

import numpy as np

S = 3
B = 8
K = 11
C = 23
P = 128
F = 512  # 256*256 = 128*512 plane layout
NACC = 12  # 3 heat + 3 ln1mp + 3 g*dd + 3 label cols

_CACHE = {}


def _build_nc():
    import concourse.bass as bass
    import concourse.bacc as bacc
    import concourse.mybir as mybir
    import concourse.tile as tile

    dt = mybir.dt
    f32, i32 = dt.float32, dt.int32
    Alu = mybir.AluOpType
    Act = mybir.ActivationFunctionType
    AX = mybir.AxisListType.X

    # Bacc (not raw Bass): its compile pipeline splits multi-wait sync into
    # event semaphores (TRN2 allows one wait per instruction)
    nc = bacc.Bacc("TRN2", target_bir_lowering=False, debug=False)
    cp = nc.declare_dram_parameter("cp", [S, C, P, F], f32, isOutput=False)
    hm = nc.declare_dram_parameter("hm", [S, K, P, F], f32, isOutput=False)
    mk = nc.declare_dram_parameter("mk", [S, P, F], f32, isOutput=False)
    lab = nc.declare_dram_parameter("lab", [K, 7], f32, isOutput=False)
    wmp = nc.declare_dram_parameter("wm", [NACC, 9], f32, isOutput=False)
    idp = nc.declare_dram_parameter("ident", [128, 128], f32, isOutput=False)
    iop = nc.declare_dram_parameter("iotap", [K, 128], f32, isOutput=False)
    iof = nc.declare_dram_parameter("iotaf", [K, F], f32, isOutput=False)
    k1p = nc.declare_dram_parameter("k128", [K, 1], f32, isOutput=False)
    cvp = nc.declare_dram_parameter("cvec", [K, 8], f32, isOutput=False)
    out = nc.declare_dram_parameter("out", [1, 16], f32, isOutput=True)

    hm_flat = hm[:].rearrange("s k p f -> (s k p) f")     # 512-wide rows
    cp_pix = cp[:].rearrange("s c p (f one) -> (s c p f) one", one=1)  # width-1 pixel rows

    with tile.TileContext(nc) as tc:
        with (
            tc.tile_pool(name="const", bufs=1) as cst,
            tc.tile_pool(name="accp", bufs=1) as accp,
            tc.tile_pool(name="big", bufs=3) as big,
            tc.tile_pool(name="sm", bufs=2) as sm,
            tc.tile_pool(name="ps", bufs=2, space="PSUM") as ps,
        ):
            # ---------------- constants (host-provided) ----------------
            ident_d = cst.tile([128, 128], f32)
            nc.sync.dma_start(out=ident_d[:], in_=idp[:])
            ident = cst.tile([128, 128], f32)
            nc.vector.tensor_copy(ident[:], ident_d[:])
            iotaPf = cst.tile([K, 128], f32)
            nc.sync.dma_start(out=iotaPf[:], in_=iop[:])
            iotaFf = cst.tile([K, F], f32)
            nc.sync.dma_start(out=iotaFf[:], in_=iof[:])
            k128f = cst.tile([K, 1], f32)
            nc.sync.dma_start(out=k128f[:], in_=k1p[:])
            cvec = cst.tile([K, 8], f32)
            nc.sync.dma_start(out=cvec[:], in_=cvp[:])
            ones = cst.tile([128, 1], f32)
            nc.vector.memset(ones[:], 1.0)
            Wm_d = cst.tile([NACC, 9], f32)
            nc.sync.dma_start(out=Wm_d[:], in_=wmp[:])
            Wm = cst.tile([NACC, 9], f32)
            nc.vector.tensor_copy(Wm[:], Wm_d[:])
            labsb = cst.tile([K, 7], f32)
            nc.sync.dma_start(out=labsb[:], in_=lab[:])

            acc = accp.tile([128, NACC], f32)
            nc.vector.memset(acc[:], 0.0)

            # ---------------- per-stack main loop ----------------
            for s in range(S):
                pred = big.tile([P, K, F], f32, tag="pred")
                gt = big.tile([P, K, F], f32, tag="gt")
                mask = big.tile([P, F], f32, tag="mask")
                mpred = big.tile([P, F], f32, tag="mpred")
                nc.sync.dma_start(out=gt[:], in_=hm[s].rearrange("k p f -> p k f"))
                nc.sync.dma_start(out=pred[:], in_=cp[s, K:2 * K].rearrange("k p f -> p k f"))
                nc.sync.dma_start(out=mask[:], in_=mk[s])
                nc.sync.dma_start(out=mpred[:], in_=cp[s, 2 * K])

                # ---- heatmap loss: sum_{k,pix} (pred*mask - gt)^2, batched
                mask_b = mask[:].rearrange("p (a f) -> p a f", a=1).to_broadcast([P, K, F])
                nc.vector.tensor_tensor(out=pred[:], in0=pred[:], in1=mask_b, op=Alu.mult)
                nc.vector.tensor_tensor(out=pred[:], in0=pred[:], in1=gt[:], op=Alu.subtract)
                nc.scalar.activation(out=pred[:], in_=pred[:], func=Act.Square,
                                     accum_out=acc[:, s:s + 1])

                # ---- mask loss: BCE(mpred, mask) summed
                ln1_m = big.tile([P, F], f32, tag="ln1m")
                lnp_m = big.tile([P, F], f32, tag="lnpm")
                nc.scalar.activation(out=ln1_m[:], in_=mpred[:], func=Act.Ln,
                                     bias=1.0, scale=-1.0,
                                     accum_out=acc[:, 3 + s:4 + s])
                nc.scalar.activation(out=lnp_m[:], in_=mpred[:], func=Act.Ln)
                nc.gpsimd.tensor_tensor(out=lnp_m[:], in0=lnp_m[:], in1=ln1_m[:],
                                        op=Alu.subtract)
                nc.vector.scalar_tensor_tensor(out=lnp_m[:], in0=lnp_m[:],
                                               scalar=0.0, in1=mask[:],
                                               op0=Alu.bypass, op1=Alu.mult,
                                               accum_out=acc[:, 6 + s:7 + s])

                # ---- label loss: per-plane argmax + gathers + BCE
                rowmax = sm.tile([P, K], f32, tag="rowmax")
                nc.vector.tensor_reduce(out=rowmax[:], in_=gt[:], axis=AX, op=Alu.max)
                pt = ps.tile([K, 128], f32, tag="pt")
                nc.tensor.transpose(out=pt[:], in_=rowmax[:], identity=ident[:])
                rowmaxT = sm.tile([K, 128], f32, tag="rmT")
                nc.vector.tensor_copy(rowmaxT[:], pt[:])
                Mx = sm.tile([K, 1], f32, tag="Mx")
                nc.vector.tensor_reduce(out=Mx[:], in_=rowmaxT[:], axis=AX, op=Alu.max)
                onehotT = sm.tile([K, 128], f32, tag="oh")
                nc.vector.tensor_scalar(out=onehotT[:], in0=rowmaxT[:],
                                        scalar1=Mx[:, 0:1], scalar2=None,
                                        op0=Alu.is_equal)
                scrT = sm.tile([K, 128], f32, tag="scrT")
                pstarf = sm.tile([K, 1], f32, tag="pstar")
                nc.vector.scalar_tensor_tensor(out=scrT[:], in0=onehotT[:],
                                               scalar=0.0, in1=iotaPf[:],
                                               op0=Alu.bypass, op1=Alu.mult,
                                               accum_out=pstarf[:])
                # winning gt row (row index = s*1408 + k*128 + p*)
                idxg_f = sm.tile([K, 1], f32, tag="idxgf")
                nc.vector.scalar_tensor_tensor(out=idxg_f[:], in0=pstarf[:],
                                               scalar=float(s * K * 128), in1=k128f[:],
                                               op0=Alu.add, op1=Alu.add)
                idxg_i = sm.tile([K, 1], i32, tag="idxgi")
                nc.vector.tensor_copy(idxg_i[:], idxg_f[:])
                grow = sm.tile([K, F], f32, tag="grow")
                nc.gpsimd.indirect_dma_start(
                    out=grow[:], out_offset=None, in_=hm_flat,
                    in_offset=bass.IndirectOffsetOnAxis(ap=idxg_i[:, 0:1], axis=0))
                wsel = sm.tile([K, F], f32, tag="wsel")
                nc.vector.tensor_scalar(out=wsel[:], in0=grow[:], scalar1=Mx[:, 0:1],
                                        scalar2=None, op0=Alu.is_equal)
                valid = sm.tile([K, 1], f32, tag="valid")
                nc.vector.tensor_scalar(out=valid[:], in0=Mx[:], scalar1=1.0,
                                        scalar2=None, op0=Alu.is_equal)
                # f* (column of max within the row), then flat pixel index
                scrF = sm.tile([K, F], f32, tag="scrF")
                fstar = sm.tile([K, 1], f32, tag="fstar")
                nc.vector.scalar_tensor_tensor(out=scrF[:], in0=wsel[:],
                                               scalar=0.0, in1=iotaFf[:],
                                               op0=Alu.bypass, op1=Alu.mult,
                                               accum_out=fstar[:])
                fidx = sm.tile([K, 1], f32, tag="fidx")
                nc.vector.scalar_tensor_tensor(out=fidx[:], in0=pstarf[:],
                                               scalar=512.0, in1=fstar[:],
                                               op0=Alu.mult, op1=Alu.add)
                # 8 flat element indices per k: (s*C + c)*65536 + p**512 + f*
                idx8f = sm.tile([K, 8], f32, tag="idx8f")
                nc.vector.scalar_tensor_tensor(
                    out=idx8f[:], in0=fidx[:, 0:1].to_broadcast([K, 8]),
                    scalar=float(s * C * 65536), in1=cvec[:],
                    op0=Alu.add, op1=Alu.add)
                idx8i = sm.tile([K, 8], i32, tag="idx8i")
                nc.vector.tensor_copy(idx8i[:], idx8f[:])
                G8 = sm.tile([K, 8], f32, tag="G8")
                for c in range(7):
                    nc.gpsimd.indirect_dma_start(
                        out=G8[:, c:c + 1], out_offset=None, in_=cp_pix,
                        in_offset=bass.IndirectOffsetOnAxis(ap=idx8i[:, c:c + 1],
                                                            axis=0))
                # BCE over gathered [K,7]
                G = G8[:, 0:7]
                lnp = sm.tile([K, 7], f32, tag="lnp")
                ln1 = sm.tile([K, 7], f32, tag="ln1")
                l1s = sm.tile([K, 1], f32, tag="l1s")
                nc.scalar.activation(out=ln1[:], in_=G, func=Act.Ln,
                                     bias=1.0, scale=-1.0, accum_out=l1s[:])
                nc.scalar.activation(out=lnp[:], in_=G, func=Act.Ln)
                dd = sm.tile([K, 7], f32, tag="dd")
                nc.vector.tensor_tensor(out=dd[:], in0=lnp[:], in1=ln1[:], op=Alu.subtract)
                scr7 = sm.tile([K, 7], f32, tag="scr7")
                wsum = sm.tile([K, 1], f32, tag="wsum")
                nc.vector.tensor_tensor(out=scr7[:], in0=dd[:], in1=labsb[:],
                                        op=Alu.mult)
                nc.vector.tensor_reduce(out=wsum[:], in_=scr7[:], axis=AX, op=Alu.add)
                tsum = sm.tile([K, 1], f32, tag="tsum")
                nc.vector.tensor_tensor(out=tsum[:], in0=wsum[:], in1=l1s[:], op=Alu.add)
                nc.vector.tensor_tensor(out=acc[0:K, 9 + s:10 + s], in0=tsum[:],
                                        in1=valid[:], op=Alu.mult)

            # ---------------- final reduction ----------------
            # stage acc through DVE so the matmul sees few producers
            acc2 = accp.tile([128, NACC], f32)
            nc.vector.tensor_copy(acc2[:], acc[:])
            ps1 = ps.tile([NACC, 1], f32, tag="ps1")
            nc.tensor.matmul(out=ps1[:], lhsT=acc2[:], rhs=ones[:], start=True, stop=True)
            s1 = sm.tile([NACC, 1], f32, tag="s1")
            nc.vector.tensor_copy(s1[:], ps1[:])
            ps2 = ps.tile([1, 9], f32, tag="ps2")
            nc.tensor.matmul(out=ps2[:], lhsT=s1[:], rhs=Wm[:], start=True, stop=True)
            res = sm.tile([1, 16], f32, tag="res")
            nc.vector.memset(res[:], 0.0)
            nc.vector.tensor_copy(res[0:1, 0:9], ps2[:])
            nc.sync.dma_start(out=out[:], in_=res[:])

    nc.finalize()
    return nc


def get_nc():
    if "nc" not in _CACHE:
        _CACHE["nc"] = _build_nc()
    return _CACHE["nc"]


def _make_wm():
    wm = np.zeros((NACC, 9), dtype=np.float32)
    for s in range(S):
        wm[s, s] = 1.0 / 11.0                # heat: accum is sum over K,pix
        wm[3 + s, 3 + s] = -1.0 / 65536.0    # mask: -(A+B)/HW
        wm[6 + s, 3 + s] = -1.0 / 65536.0
        wm[9 + s, 6 + s] = -1.0 / 77.0       # label: -sum/(7*11)
    return wm


def make_in_maps(combined_preds, heatmaps, labels, masks):
    cpn = np.asarray(combined_preds, dtype=np.float32)
    hmn = np.asarray(heatmaps, dtype=np.float32)
    lbn = np.asarray(labels, dtype=np.float32)
    mkn = np.asarray(masks, dtype=np.float32)
    wm = _make_wm()
    ident = np.eye(128, dtype=np.float32)
    iotap = np.broadcast_to(np.arange(128, dtype=np.float32), (K, 128)).copy()
    iotaf = np.broadcast_to(np.arange(F, dtype=np.float32), (K, F)).copy()
    k128 = (np.arange(K, dtype=np.float32) * 128.0).reshape(K, 1)
    cvec = np.broadcast_to(np.arange(8, dtype=np.float32) * 65536.0, (K, 8)).copy()
    in_maps = []
    for b in range(B):
        in_maps.append({
            "cp": np.ascontiguousarray(cpn[:, b]).reshape(S, C, P, F),
            "hm": np.ascontiguousarray(hmn[:, b]).reshape(S, K, P, F),
            "mk": np.ascontiguousarray(mkn[:, b, 0]).reshape(S, P, F),
            "lab": np.ascontiguousarray(lbn[b]),
            "wm": wm,
            "ident": ident,
            "iotap": iotap,
            "iotaf": iotaf,
            "k128": k128,
            "cvec": cvec,
        })
    return in_maps


def run_spmd(in_maps, trace=False, **kw):
    from concourse.bass_utils import run_bass_kernel_spmd
    return run_bass_kernel_spmd(get_nc(), in_maps, core_ids=list(range(B)),
                                trace=trace, **kw)


def kernel(combined_preds, heatmaps, labels, masks):
    res = run_spmd(make_in_maps(combined_preds, heatmaps, labels, masks)).results
    heat = np.stack([res[b]["out"][0, 0:3] for b in range(B)]).astype(np.float32)
    mask_l = np.stack([res[b]["out"][0, 3:6] for b in range(B)]).astype(np.float32)
    label = np.stack([res[b]["out"][0, 6:9] for b in range(B)]).astype(np.float32)
    return (heat, label, mask_l)



# revision 7
# speedup vs baseline: 1.0908x; 1.0908x over previous
"""Trainium2 Bass kernel for nn_KeypointLoss (S=3, B=8, K=11, C=23, H=W=256).

Data-parallel over batch B=8 across 8 NeuronCores: core b computes the three
losses (heatmap / label / mask) for batch element b; host assembles [B,S].

Per-core device algorithm (v2, DMA/DVE balanced):
  loads : pred planes on the sync queue, gt planes on the gpsimd queue
          (two DMA queues in parallel), masks + consts on the PE queue.
  heat  : pm = pred*mask (DVE f32->bf16), gtb = cast(gt) on ACT, d = pm-gtb
          (DVE bf16 2x), ACT Square+accum -> acc col s
  label : rowmax on bf16 (DVE 2x) + PE transpose; one indirect row-fetch per
          stack to recover the argmax column; the 7 label-channel pixels per
          keypoint gathered with one [77,1] indirect DMA per stack whose
          offset table is built by a tiny PE matmul; BCE on [77,3]
  mask  : BCE via ACT Ln(+accum) and two small DVE ops (not gpsimd)
  final : two small matmuls reduce partition partials -> out[1,16]
"""

import numpy as np

S = 3
B = 8
K = 11
C = 23
P = 128
F = 512  # 256*256 = 128*512 plane layout
NACC = 12  # 3 heat + 3 ln1mp + 3 g*dd + 3 label cols

_CACHE = {}


def _build_nc():
    import concourse.bass as bass
    import concourse.bacc as bacc
    import concourse.mybir as mybir
    import concourse.tile as tile

    dt = mybir.dt
    f32, i32, bf16 = dt.float32, dt.int32, dt.bfloat16
    Alu = mybir.AluOpType
    Act = mybir.ActivationFunctionType
    AX = mybir.AxisListType.X

    nc = bacc.Bacc("TRN2", target_bir_lowering=False, debug=False)
    cp = nc.declare_dram_parameter("cp", [S, C, P, F], f32, isOutput=False)
    hm = nc.declare_dram_parameter("hm", [S, K, P, F], f32, isOutput=False)
    mk = nc.declare_dram_parameter("mk", [S, P, F], f32, isOutput=False)
    lab = nc.declare_dram_parameter("lab", [K, 7], f32, isOutput=False)
    wmp = nc.declare_dram_parameter("wm", [NACC, 9], f32, isOutput=False)
    idp = nc.declare_dram_parameter("identb", [128, 128], bf16, isOutput=False)
    iop = nc.declare_dram_parameter("iotap", [K, 128], f32, isOutput=False)
    iof = nc.declare_dram_parameter("iotaf", [K, F], f32, isOutput=False)
    skp = nc.declare_dram_parameter("sk113", [K, 3], f32, isOutput=False)
    m77p = nc.declare_dram_parameter("m77k", [K, 77], f32, isOutput=False)
    cvp = nc.declare_dram_parameter("cvecs77", [77, 3], f32, isOutput=False)
    selp = nc.declare_dram_parameter("sel77", [77, K], f32, isOutput=False)
    out = nc.declare_dram_parameter("out", [1, 16], f32, isOutput=True)

    hm_flat = hm[:].rearrange("s k p f -> (s k p) f")     # 512-wide rows
    cp_pix = cp[:].rearrange("s c p (f one) -> (s c p f) one", one=1)
    lab77v = lab[:].rearrange("k (c one) -> (k c) one", one=1)

    with tile.TileContext(nc) as tc:
        with (
            tc.tile_pool(name="const", bufs=1) as cst,
            tc.tile_pool(name="accp", bufs=1) as accp,
            tc.tile_pool(name="big", bufs=2) as big,
            tc.tile_pool(name="sm", bufs=1) as sm,
            tc.tile_pool(name="ps", bufs=2, space="PSUM") as ps,
        ):
            # ------------- constants (PE queue) + memsets -------------
            identb = cst.tile([128, 128], bf16)
            nc.scalar.dma_start(out=identb[:], in_=idp[:])
            iotaP = cst.tile([K, 128], f32)
            nc.scalar.dma_start(out=iotaP[:], in_=iop[:])
            iotaF = cst.tile([K, F], f32)
            nc.scalar.dma_start(out=iotaF[:], in_=iof[:])
            sk113 = cst.tile([K, 3], f32)
            nc.scalar.dma_start(out=sk113[:], in_=skp[:])
            m77k = cst.tile([K, 77], f32)
            nc.scalar.dma_start(out=m77k[:], in_=m77p[:])
            cvecs77 = cst.tile([77, 3], f32)
            nc.scalar.dma_start(out=cvecs77[:], in_=cvp[:])
            sel77 = cst.tile([77, K], f32)
            nc.scalar.dma_start(out=sel77[:], in_=selp[:])
            lab77 = cst.tile([77, 1], f32)
            nc.scalar.dma_start(out=lab77[:], in_=lab77v)
            Wm = cst.tile([NACC, 9], f32)
            nc.scalar.dma_start(out=Wm[:], in_=wmp[:])
            ones = cst.tile([128, 1], f32)
            nc.vector.memset(ones[:], 1.0)

            acc = accp.tile([128, NACC], f32)
            nc.vector.memset(acc[:], 0.0)

            # ------------- input loads: two big queues -------------
            preds, gts, masks, mpreds = [], [], [], []
            for s in range(S):
                pred = big.tile([P, K, F], f32, tag="pred")
                gt = big.tile([P, K, F], f32, tag="gt")
                mask = big.tile([P, F], f32, tag="mask", bufs=3)
                mpred = big.tile([P, F], f32, tag="mpred")
                nc.sync.dma_start(out=pred[:], in_=cp[s, K:2 * K].rearrange("k p f -> p k f"))
                nc.gpsimd.dma_start(out=gt[:], in_=hm[s].rearrange("k p f -> p k f"))
                nc.scalar.dma_start(out=mask[:], in_=mk[s])
                nc.scalar.dma_start(out=mpred[:], in_=cp[s, 2 * K])
                preds.append(pred); gts.append(gt); masks.append(mask); mpreds.append(mpred)

            # chain-A result tiles shared across stacks (column s each)
            Mx113 = sm.tile([K, 3], f32)
            pstar113 = sm.tile([K, 3], f32)
            rowmaxs, pts = [], []

            # ------------- per-stack heavy compute -------------
            for s in range(S):
                pred, gt, mask, mpred = preds[s], gts[s], masks[s], mpreds[s]
                # heat: pm = pred*mask (bf16 out), gtb = cast(gt), d = pm-gtb
                pm = big.tile([P, K, F], bf16, tag="pm")
                mask_b = mask[:].rearrange("p (a f) -> p a f", a=1).to_broadcast([P, K, F])
                nc.vector.tensor_tensor(out=pm[:], in0=pred[:], in1=mask_b, op=Alu.mult)
                gtb = big.tile([P, K, F], bf16, tag="gtb")
                nc.scalar.activation(out=gtb[:], in_=gt[:], func=Act.Copy)
                rowmax = sm.tile([P, K], bf16, tag="rowmax", bufs=3)
                nc.vector.tensor_reduce(out=rowmax[:], in_=gtb[:], axis=AX, op=Alu.max)
                nc.vector.tensor_tensor(out=pm[:], in0=pm[:], in1=gtb[:], op=Alu.subtract)
                pt = ps.tile([K, 128], bf16, tag="pt", bufs=2)
                nc.tensor.transpose(out=pt[:], in_=rowmax[:], identity=identb[:])
                rmT = sm.tile([K, 128], f32, tag="rmT", bufs=2)
                nc.scalar.activation(out=rmT[:], in_=pt[:], func=Act.Copy)
                # label chain A: global max + argmax partition (column s)
                nc.vector.tensor_reduce(out=Mx113[:, s:s + 1], in_=rmT[:], axis=AX, op=Alu.max)
                oh = sm.tile([K, 128], f32, tag="oh", bufs=2)
                nc.vector.tensor_scalar(out=oh[:], in0=rmT[:], scalar1=Mx113[:, s:s + 1],
                                        scalar2=None, op0=Alu.is_equal)
                scrP = sm.tile([K, 128], f32, tag="scrP", bufs=2)
                nc.vector.scalar_tensor_tensor(out=scrP[:], in0=oh[:], scalar=0.0,
                                               in1=iotaP[:], op0=Alu.bypass, op1=Alu.mult,
                                               accum_out=pstar113[:, s:s + 1])
                # mask loss (ACT lns first, then square which is needed late)
                ln1m = big.tile([P, F], f32, tag="ln1m")
                lnp = big.tile([P, F], f32, tag="lnp")
                nc.scalar.activation(out=ln1m[:], in_=mpred[:], func=Act.Ln,
                                     bias=1.0, scale=-1.0, accum_out=acc[:, 3 + s:4 + s])
                nc.scalar.activation(out=lnp[:], in_=mpred[:], func=Act.Ln)
                nc.scalar.activation(out=pm[:], in_=pm[:], func=Act.Square,
                                     accum_out=acc[:, s:s + 1])
                ddm = big.tile([P, F], f32, tag="ddm")
                nc.vector.tensor_tensor(out=ddm[:], in0=lnp[:], in1=ln1m[:], op=Alu.subtract)
                nc.vector.scalar_tensor_tensor(out=ddm[:], in0=ddm[:], scalar=0.0,
                                               in1=mask[:], op0=Alu.bypass, op1=Alu.mult,
                                               accum_out=acc[:, 6 + s:7 + s])

            # ------------- label chain B (all stacks) -------------
            pstar_c = sm.tile([K, 3], f32)
            nc.vector.tensor_scalar_min(pstar_c[:], pstar113[:], 127.0)
            valid113 = sm.tile([K, 3], f32)
            nc.vector.tensor_scalar(out=valid113[:], in0=Mx113[:], scalar1=1.0,
                                    scalar2=None, op0=Alu.is_equal)
            idxg_i = sm.tile([K, 3], i32)
            nc.vector.tensor_tensor(out=idxg_i[:], in0=pstar_c[:], in1=sk113[:], op=Alu.add)
            grow3 = sm.tile([K, 3, F], f32)
            for s in range(S):
                nc.gpsimd.indirect_dma_start(
                    out=grow3[:, s, :], out_offset=None, in_=hm_flat,
                    in_offset=bass.IndirectOffsetOnAxis(ap=idxg_i[:, s:s + 1], axis=0))
            fstar113 = sm.tile([K, 3], f32)
            for s in range(S):
                wsel = sm.tile([K, F], f32, tag="wsel", bufs=2)
                nc.vector.tensor_scalar(out=wsel[:], in0=grow3[:, s, :],
                                        scalar1=Mx113[:, s:s + 1], scalar2=None,
                                        op0=Alu.is_equal)
                nc.vector.scalar_tensor_tensor(out=wsel[:], in0=wsel[:], scalar=0.0,
                                               in1=iotaF[:], op0=Alu.bypass, op1=Alu.mult,
                                               accum_out=fstar113[:, s:s + 1])
            fstar_c = sm.tile([K, 3], f32)
            nc.vector.tensor_scalar_min(fstar_c[:], fstar113[:], 511.0)
            fidx113 = sm.tile([K, 3], f32)
            nc.vector.scalar_tensor_tensor(out=fidx113[:], in0=pstar_c[:], scalar=512.0,
                                           in1=fstar_c[:], op0=Alu.mult, op1=Alu.add)
            ps_idx = ps.tile([77, 3], f32, tag="psidx", bufs=1)
            nc.tensor.matmul(out=ps_idx[:], lhsT=m77k[:], rhs=fidx113[:], start=True, stop=True)
            idx77i = sm.tile([77, 3], i32)
            nc.vector.tensor_tensor(out=idx77i[:], in0=ps_idx[:], in1=cvecs77[:], op=Alu.add)
            G77 = sm.tile([77, 3], f32)
            for s in range(S):
                nc.gpsimd.indirect_dma_start(
                    out=G77[:, s:s + 1], out_offset=None, in_=cp_pix,
                    in_offset=bass.IndirectOffsetOnAxis(ap=idx77i[:, s:s + 1], axis=0))
            ln1m77 = sm.tile([77, 3], f32)
            lnp77 = sm.tile([77, 3], f32)
            nc.scalar.activation(out=ln1m77[:], in_=G77[:], func=Act.Ln,
                                 bias=1.0, scale=-1.0)
            nc.scalar.activation(out=lnp77[:], in_=G77[:], func=Act.Ln)
            dd77 = sm.tile([77, 3], f32)
            nc.vector.tensor_tensor(out=dd77[:], in0=lnp77[:], in1=ln1m77[:], op=Alu.subtract)
            nc.vector.tensor_scalar(out=dd77[:], in0=dd77[:], scalar1=lab77[:, 0:1],
                                    scalar2=None, op0=Alu.mult)
            bce77 = sm.tile([77, 3], f32)
            nc.vector.tensor_tensor(out=bce77[:], in0=dd77[:], in1=ln1m77[:], op=Alu.add)
            lbl_ps = ps.tile([K, 3], f32, tag="lblps", bufs=1)
            nc.tensor.matmul(out=lbl_ps[:], lhsT=sel77[:], rhs=bce77[:], start=True, stop=True)
            for s in range(S):
                nc.vector.tensor_tensor(out=acc[0:K, 9 + s:10 + s],
                                        in0=lbl_ps[:, s:s + 1],
                                        in1=valid113[:, s:s + 1], op=Alu.mult)

            # ------------- final reduction -------------
            acc2 = accp.tile([128, NACC], f32)
            nc.vector.tensor_copy(acc2[:], acc[:])
            ps1 = ps.tile([NACC, 1], f32, tag="ps1", bufs=1)
            nc.tensor.matmul(out=ps1[:], lhsT=acc2[:], rhs=ones[:], start=True, stop=True)
            s1 = sm.tile([NACC, 1], f32)
            nc.vector.tensor_copy(s1[:], ps1[:])
            ps2 = ps.tile([1, 9], f32, tag="ps2", bufs=1)
            nc.tensor.matmul(out=ps2[:], lhsT=s1[:], rhs=Wm[:], start=True, stop=True)
            res = sm.tile([1, 16], f32)
            nc.vector.memset(res[:], 0.0)
            nc.vector.tensor_copy(res[0:1, 0:9], ps2[:])
            nc.sync.dma_start(out=out[:], in_=res[:])

    nc.finalize()
    return nc


def get_nc():
    if "nc" not in _CACHE:
        _CACHE["nc"] = _build_nc()
    return _CACHE["nc"]


def _make_wm():
    wm = np.zeros((NACC, 9), dtype=np.float32)
    for s in range(S):
        wm[s, s] = 1.0 / 11.0                # heat: accum is sum over K,pix
        wm[3 + s, 3 + s] = -1.0 / 65536.0    # mask: -(A+B)/HW
        wm[6 + s, 3 + s] = -1.0 / 65536.0
        wm[9 + s, 6 + s] = -1.0 / 77.0       # label: -sum/(7*11)
    return wm


def _consts():
    if "consts" in _CACHE:
        return _CACHE["consts"]
    import ml_dtypes
    ident = np.eye(128, dtype=np.float32).astype(ml_dtypes.bfloat16)
    iotap = np.broadcast_to(np.arange(128, dtype=np.float32), (K, 128)).copy()
    iotaf = np.broadcast_to(np.arange(F, dtype=np.float32), (K, F)).copy()
    ks = np.arange(K, dtype=np.float32)[:, None] * 128.0
    ss = np.arange(S, dtype=np.float32)[None, :] * (K * 128.0)
    sk113 = (ks + ss).astype(np.float32)                      # [K,3]
    r = np.arange(77)
    m77k = np.zeros((K, 77), dtype=np.float32)
    m77k[r // 7, r] = 1.0                                     # [K,77] lhsT
    cvecs77 = ((r % 7)[:, None] * 65536.0 +
               np.arange(S)[None, :] * (C * 65536.0)).astype(np.float32)  # [77,3]
    sel77 = np.zeros((77, K), dtype=np.float32)
    sel77[r, r // 7] = 1.0                                    # [77,K] lhsT
    _CACHE["consts"] = dict(wm=_make_wm(), identb=ident, iotap=iotap, iotaf=iotaf,
                            sk113=sk113, m77k=m77k, cvecs77=cvecs77, sel77=sel77)
    return _CACHE["consts"]


def make_in_maps(combined_preds, heatmaps, labels, masks):
    cpn = np.asarray(combined_preds, dtype=np.float32)
    hmn = np.asarray(heatmaps, dtype=np.float32)
    lbn = np.asarray(labels, dtype=np.float32)
    mkn = np.asarray(masks, dtype=np.float32)
    cc = _consts()
    in_maps = []
    for b in range(B):
        m = {
            "cp": np.ascontiguousarray(cpn[:, b]).reshape(S, C, P, F),
            "hm": np.ascontiguousarray(hmn[:, b]).reshape(S, K, P, F),
            "mk": np.ascontiguousarray(mkn[:, b, 0]).reshape(S, P, F),
            "lab": np.ascontiguousarray(lbn[b]),
        }
        m.update(cc)
        in_maps.append(m)
    return in_maps


def run_spmd(in_maps, trace=False, **kw):
    from concourse.bass_utils import run_bass_kernel_spmd
    return run_bass_kernel_spmd(get_nc(), in_maps, core_ids=list(range(B)),
                                trace=trace, **kw)


def kernel(combined_preds, heatmaps, labels, masks):
    res = run_spmd(make_in_maps(combined_preds, heatmaps, labels, masks)).results
    heat = np.stack([res[b]["out"][0, 0:3] for b in range(B)]).astype(np.float32)
    mask_l = np.stack([res[b]["out"][0, 3:6] for b in range(B)]).astype(np.float32)
    label = np.stack([res[b]["out"][0, 6:9] for b in range(B)]).astype(np.float32)
    return (heat, label, mask_l)


# revision 8
# speedup vs baseline: 1.2192x; 1.1178x over previous
"""Trainium2 Bass kernel for nn_KeypointLoss (S=3, B=8, K=11, C=23, H=W=256).

Data-parallel over batch B=8 across 8 NeuronCores: core b computes the three
losses (heatmap / label / mask) for batch element b; host assembles [B,S].

Per-core device algorithm (v3, pipeline-ordered):
  loads : all 12 plane loads on the sync queue in stack order
          [gt_s, pred_s, mask_s, mpred_s] so stack s's compute overlaps
          stack s+1's loads; the 9 small consts go on the scalar queue.
  heat  : gtb = cast(gt) on ACT (f32->bf16), pm = pred*mask (DVE, bf16 out),
          d = pm-gtb (DVE bf16 2x), ACT Square+accum -> acc col s
  label : rowmax in two steps (bf16 TT max on halves @2x + small reduce),
          PE transpose; per-stack indirect row-fetch recovers the argmax
          column; the 7 label-channel pixels per keypoint are gathered with
          one [77,1] indirect DMA per stack whose offset table is built by a
          tiny PE matmul; BCE batched on [77,3]
  mask  : BCE via ACT Ln(+accum) and two small DVE ops
  final : two small matmuls reduce partition partials -> out[1,16]
"""

import numpy as np

S = 3
B = 8
K = 11
C = 23
P = 128
F = 512  # 256*256 = 128*512 plane layout
NACC = 12  # 3 heat + 3 ln1mp + 3 g*dd + 3 label cols

_CACHE = {}


def _build_nc():
    import concourse.bass as bass
    import concourse.bacc as bacc
    import concourse.mybir as mybir
    import concourse.tile as tile

    dt = mybir.dt
    f32, i32, bf16 = dt.float32, dt.int32, dt.bfloat16
    Alu = mybir.AluOpType
    Act = mybir.ActivationFunctionType
    AX = mybir.AxisListType.X

    nc = bacc.Bacc("TRN2", target_bir_lowering=False, debug=False)
    cp = nc.declare_dram_parameter("cp", [S, C, P, F], f32, isOutput=False)
    hm = nc.declare_dram_parameter("hm", [S, K, P, F], f32, isOutput=False)
    mk = nc.declare_dram_parameter("mk", [S, P, F], f32, isOutput=False)
    lab = nc.declare_dram_parameter("lab", [K, 7], f32, isOutput=False)
    wmp = nc.declare_dram_parameter("wm", [NACC, 9], f32, isOutput=False)
    idp = nc.declare_dram_parameter("identb", [128, 128], bf16, isOutput=False)
    iop = nc.declare_dram_parameter("iotap", [K, 128], f32, isOutput=False)
    iof = nc.declare_dram_parameter("iotaf", [K, F], f32, isOutput=False)
    skp = nc.declare_dram_parameter("sk113", [K, 3], f32, isOutput=False)
    m77p = nc.declare_dram_parameter("m77k", [K, 77], f32, isOutput=False)
    cvp = nc.declare_dram_parameter("cvecs77", [77, 3], f32, isOutput=False)
    selp = nc.declare_dram_parameter("sel77", [77, K], f32, isOutput=False)
    out = nc.declare_dram_parameter("out", [1, 16], f32, isOutput=True)

    hm_flat = hm[:].rearrange("s k p f -> (s k p) f")     # 512-wide rows
    cp_pix = cp[:].rearrange("s c p (f one) -> (s c p f) one", one=1)
    lab77v = lab[:].rearrange("k (c one) -> (k c) one", one=1)

    with tile.TileContext(nc) as tc:
        with (
            tc.tile_pool(name="const", bufs=1) as cst,
            tc.tile_pool(name="accp", bufs=1) as accp,
            tc.tile_pool(name="big", bufs=2) as big,
            tc.tile_pool(name="sm", bufs=1) as sm,
            tc.tile_pool(name="ps", bufs=2, space="PSUM") as ps,
        ):
            # ------------- constants (scalar queue) + memsets -------------
            identb = cst.tile([128, 128], bf16)
            nc.scalar.dma_start(out=identb[:], in_=idp[:])
            iotaP = cst.tile([K, 128], f32)
            nc.scalar.dma_start(out=iotaP[:], in_=iop[:])
            iotaF = cst.tile([K, F], f32)
            nc.scalar.dma_start(out=iotaF[:], in_=iof[:])
            sk113 = cst.tile([K, 3], f32)
            nc.scalar.dma_start(out=sk113[:], in_=skp[:])
            m77k = cst.tile([K, 77], f32)
            nc.scalar.dma_start(out=m77k[:], in_=m77p[:])
            cvecs77 = cst.tile([77, 3], f32)
            nc.scalar.dma_start(out=cvecs77[:], in_=cvp[:])
            sel77 = cst.tile([77, K], f32)
            nc.scalar.dma_start(out=sel77[:], in_=selp[:])
            lab77 = cst.tile([77, 1], f32)
            nc.scalar.dma_start(out=lab77[:], in_=lab77v)
            Wm = cst.tile([NACC, 9], f32)
            nc.scalar.dma_start(out=Wm[:], in_=wmp[:])
            ones = cst.tile([128, 1], f32)
            nc.vector.memset(ones[:], 1.0)

            acc = accp.tile([128, NACC], f32)
            nc.vector.memset(acc[:], 0.0)

            # ------------- input loads: one ordered sync queue -------------
            preds, gts, masks, mpreds = [], [], [], []
            for s in range(S):
                gt = big.tile([P, K, F], f32, tag="gt")
                pred = big.tile([P, K, F], f32, tag="pred")
                mask = big.tile([P, F], f32, tag="mask", bufs=3)
                mpred = big.tile([P, F], f32, tag="mpred")
                nc.sync.dma_start(out=gt[:], in_=hm[s].rearrange("k p f -> p k f"))
                nc.sync.dma_start(out=pred[:], in_=cp[s, K:2 * K].rearrange("k p f -> p k f"))
                nc.sync.dma_start(out=mask[:], in_=mk[s])
                nc.sync.dma_start(out=mpred[:], in_=cp[s, 2 * K])
                preds.append(pred); gts.append(gt); masks.append(mask); mpreds.append(mpred)

            # chain-A result tiles shared across stacks (column s each)
            Mx113 = sm.tile([K, 3], f32)
            pstar113 = sm.tile([K, 3], f32)
            pstar_c = sm.tile([K, 3], f32)
            idxg_i = sm.tile([K, 3], i32)
            grow3 = sm.tile([K, 3, F], f32)
            pms = []

            # ------------- per-stack heavy compute -------------
            for s in range(S):
                pred, gt, mask, mpred = preds[s], gts[s], masks[s], mpreds[s]
                gtb = big.tile([P, K, F], bf16, tag="gtb")
                nc.scalar.activation(out=gtb[:], in_=gt[:], func=Act.Copy)
                # rowmax in two steps: TT max halves (bf16 2x) + small reduce
                hmax = big.tile([P, K, F // 2], bf16, tag="hmax")
                nc.vector.tensor_tensor(out=hmax[:], in0=gtb[:, :, 0:F // 2],
                                        in1=gtb[:, :, F // 2:F], op=Alu.max)
                rowmax = sm.tile([P, K], bf16, tag="rowmax", bufs=3)
                nc.vector.tensor_reduce(out=rowmax[:], in_=hmax[:], axis=AX, op=Alu.max)
                # heat: pm = pred*mask (bf16 out), d = pm - gtb
                pm = big.tile([P, K, F], bf16, tag="pm")
                mask_b = mask[:].rearrange("p (a f) -> p a f", a=1).to_broadcast([P, K, F])
                nc.vector.tensor_tensor(out=pm[:], in0=pred[:], in1=mask_b, op=Alu.mult)
                nc.vector.tensor_tensor(out=pm[:], in0=pm[:], in1=gtb[:], op=Alu.subtract)
                pms.append(pm)
                pt = ps.tile([K, 128], bf16, tag="pt", bufs=2)
                nc.tensor.transpose(out=pt[:], in_=rowmax[:], identity=identb[:])
                rmT = sm.tile([K, 128], f32, tag="rmT", bufs=2)
                nc.scalar.activation(out=rmT[:], in_=pt[:], func=Act.Copy)
                # mask loss lns (early in ACT order)
                ln1m = big.tile([P, F], f32, tag="ln1m")
                lnp = big.tile([P, F], f32, tag="lnp")
                nc.scalar.activation(out=ln1m[:], in_=mpred[:], func=Act.Ln,
                                     bias=1.0, scale=-1.0, accum_out=acc[:, 3 + s:4 + s])
                nc.scalar.activation(out=lnp[:], in_=mpred[:], func=Act.Ln)
                # label chain A: global max + argmax partition (column s)
                nc.vector.tensor_reduce(out=Mx113[:, s:s + 1], in_=rmT[:], axis=AX, op=Alu.max)
                oh = sm.tile([K, 128], f32, tag="oh", bufs=2)
                nc.vector.tensor_scalar(out=oh[:], in0=rmT[:], scalar1=Mx113[:, s:s + 1],
                                        scalar2=None, op0=Alu.is_equal)
                scrP = sm.tile([K, 128], f32, tag="scrP", bufs=2)
                nc.vector.scalar_tensor_tensor(out=scrP[:], in0=oh[:], scalar=0.0,
                                               in1=iotaP[:], op0=Alu.bypass, op1=Alu.mult,
                                               accum_out=pstar113[:, s:s + 1])
                nc.vector.tensor_scalar_min(pstar_c[:, s:s + 1], pstar113[:, s:s + 1], 127.0)
                nc.vector.tensor_tensor(out=idxg_i[:, s:s + 1], in0=pstar_c[:, s:s + 1],
                                        in1=sk113[:, s:s + 1], op=Alu.add)
                nc.gpsimd.indirect_dma_start(
                    out=grow3[:, s, :], out_offset=None, in_=hm_flat,
                    in_offset=bass.IndirectOffsetOnAxis(ap=idxg_i[:, s:s + 1], axis=0))
                # mask loss combine (DVE, small)
                ddm = big.tile([P, F], f32, tag="ddm")
                nc.vector.tensor_tensor(out=ddm[:], in0=lnp[:], in1=ln1m[:], op=Alu.subtract)
                nc.vector.scalar_tensor_tensor(out=ddm[:], in0=ddm[:], scalar=0.0,
                                               in1=mask[:], op0=Alu.bypass, op1=Alu.mult,
                                               accum_out=acc[:, 6 + s:7 + s])
                # deferred square of the PREVIOUS stack (keeps cast_s early in
                # the ACT stream; sq result is only needed at the very end)
                if s > 0:
                    nc.scalar.activation(out=pms[s - 1][:], in_=pms[s - 1][:],
                                         func=Act.Square, accum_out=acc[:, s - 1:s])
            nc.scalar.activation(out=pms[S - 1][:], in_=pms[S - 1][:],
                                 func=Act.Square, accum_out=acc[:, S - 1:S])

            # ------------- label chain B (all stacks) -------------
            valid113 = sm.tile([K, 3], f32)
            nc.vector.tensor_scalar(out=valid113[:], in0=Mx113[:], scalar1=1.0,
                                    scalar2=None, op0=Alu.is_equal)
            fstar113 = sm.tile([K, 3], f32)
            for s in range(S):
                wsel = sm.tile([K, F], f32, tag="wsel", bufs=2)
                nc.vector.tensor_scalar(out=wsel[:], in0=grow3[:, s, :],
                                        scalar1=Mx113[:, s:s + 1], scalar2=None,
                                        op0=Alu.is_equal)
                nc.vector.scalar_tensor_tensor(out=wsel[:], in0=wsel[:], scalar=0.0,
                                               in1=iotaF[:], op0=Alu.bypass, op1=Alu.mult,
                                               accum_out=fstar113[:, s:s + 1])
            fstar_c = sm.tile([K, 3], f32)
            nc.vector.tensor_scalar_min(fstar_c[:], fstar113[:], 511.0)
            fidx113 = sm.tile([K, 3], f32)
            nc.vector.scalar_tensor_tensor(out=fidx113[:], in0=pstar_c[:], scalar=512.0,
                                           in1=fstar_c[:], op0=Alu.mult, op1=Alu.add)
            ps_idx = ps.tile([77, 3], f32, tag="psidx", bufs=1)
            nc.tensor.matmul(out=ps_idx[:], lhsT=m77k[:], rhs=fidx113[:], start=True, stop=True)
            idx77i = sm.tile([77, 3], i32)
            nc.vector.tensor_tensor(out=idx77i[:], in0=ps_idx[:], in1=cvecs77[:], op=Alu.add)
            G77 = sm.tile([77, 3], f32)
            for s in range(S):
                nc.gpsimd.indirect_dma_start(
                    out=G77[:, s:s + 1], out_offset=None, in_=cp_pix,
                    in_offset=bass.IndirectOffsetOnAxis(ap=idx77i[:, s:s + 1], axis=0))
            ln1m77 = sm.tile([77, 3], f32)
            lnp77 = sm.tile([77, 3], f32)
            nc.scalar.activation(out=ln1m77[:], in_=G77[:], func=Act.Ln,
                                 bias=1.0, scale=-1.0)
            nc.scalar.activation(out=lnp77[:], in_=G77[:], func=Act.Ln)
            dd77 = sm.tile([77, 3], f32)
            nc.vector.tensor_tensor(out=dd77[:], in0=lnp77[:], in1=ln1m77[:], op=Alu.subtract)
            nc.vector.tensor_scalar(out=dd77[:], in0=dd77[:], scalar1=lab77[:, 0:1],
                                    scalar2=None, op0=Alu.mult)
            bce77 = sm.tile([77, 3], f32)
            nc.vector.tensor_tensor(out=bce77[:], in0=dd77[:], in1=ln1m77[:], op=Alu.add)
            lbl_ps = ps.tile([K, 3], f32, tag="lblps", bufs=1)
            nc.tensor.matmul(out=lbl_ps[:], lhsT=sel77[:], rhs=bce77[:], start=True, stop=True)
            for s in range(S):
                nc.vector.tensor_tensor(out=acc[0:K, 9 + s:10 + s],
                                        in0=lbl_ps[:, s:s + 1],
                                        in1=valid113[:, s:s + 1], op=Alu.mult)

            # ------------- final reduction -------------
            acc2 = accp.tile([128, NACC], f32)
            nc.vector.tensor_copy(acc2[:], acc[:])
            ps1 = ps.tile([NACC, 1], f32, tag="ps1", bufs=1)
            nc.tensor.matmul(out=ps1[:], lhsT=acc2[:], rhs=ones[:], start=True, stop=True)
            s1 = sm.tile([NACC, 1], f32)
            nc.vector.tensor_copy(s1[:], ps1[:])
            ps2 = ps.tile([1, 9], f32, tag="ps2", bufs=1)
            nc.tensor.matmul(out=ps2[:], lhsT=s1[:], rhs=Wm[:], start=True, stop=True)
            res = sm.tile([1, 16], f32)
            nc.vector.memset(res[:], 0.0)
            nc.vector.tensor_copy(res[0:1, 0:9], ps2[:])
            nc.sync.dma_start(out=out[:], in_=res[:])

    nc.finalize()
    return nc


def get_nc():
    if "nc" not in _CACHE:
        _CACHE["nc"] = _build_nc()
    return _CACHE["nc"]


def _make_wm():
    wm = np.zeros((NACC, 9), dtype=np.float32)
    for s in range(S):
        wm[s, s] = 1.0 / 11.0                # heat: accum is sum over K,pix
        wm[3 + s, 3 + s] = -1.0 / 65536.0    # mask: -(A+B)/HW
        wm[6 + s, 3 + s] = -1.0 / 65536.0
        wm[9 + s, 6 + s] = -1.0 / 77.0       # label: -sum/(7*11)
    return wm


def _consts():
    if "consts" in _CACHE:
        return _CACHE["consts"]
    import ml_dtypes
    ident = np.eye(128, dtype=np.float32).astype(ml_dtypes.bfloat16)
    iotap = np.broadcast_to(np.arange(128, dtype=np.float32), (K, 128)).copy()
    iotaf = np.broadcast_to(np.arange(F, dtype=np.float32), (K, F)).copy()
    ks = np.arange(K, dtype=np.float32)[:, None] * 128.0
    ss = np.arange(S, dtype=np.float32)[None, :] * (K * 128.0)
    sk113 = (ks + ss).astype(np.float32)                      # [K,3]
    r = np.arange(77)
    m77k = np.zeros((K, 77), dtype=np.float32)
    m77k[r // 7, r] = 1.0                                     # [K,77] lhsT
    cvecs77 = ((r % 7)[:, None] * 65536.0 +
               np.arange(S)[None, :] * (C * 65536.0)).astype(np.float32)  # [77,3]
    sel77 = np.zeros((77, K), dtype=np.float32)
    sel77[r, r // 7] = 1.0                                    # [77,K] lhsT
    _CACHE["consts"] = dict(wm=_make_wm(), identb=ident, iotap=iotap, iotaf=iotaf,
                            sk113=sk113, m77k=m77k, cvecs77=cvecs77, sel77=sel77)
    return _CACHE["consts"]


def make_in_maps(combined_preds, heatmaps, labels, masks):
    cpn = np.asarray(combined_preds, dtype=np.float32)
    hmn = np.asarray(heatmaps, dtype=np.float32)
    lbn = np.asarray(labels, dtype=np.float32)
    mkn = np.asarray(masks, dtype=np.float32)
    cc = _consts()
    in_maps = []
    for b in range(B):
        m = {
            "cp": np.ascontiguousarray(cpn[:, b]).reshape(S, C, P, F),
            "hm": np.ascontiguousarray(hmn[:, b]).reshape(S, K, P, F),
            "mk": np.ascontiguousarray(mkn[:, b, 0]).reshape(S, P, F),
            "lab": np.ascontiguousarray(lbn[b]),
        }
        m.update(cc)
        in_maps.append(m)
    return in_maps


def run_spmd(in_maps, trace=False, **kw):
    from concourse.bass_utils import run_bass_kernel_spmd
    return run_bass_kernel_spmd(get_nc(), in_maps, core_ids=list(range(B)),
                                trace=trace, **kw)


def kernel(combined_preds, heatmaps, labels, masks):
    res = run_spmd(make_in_maps(combined_preds, heatmaps, labels, masks)).results
    heat = np.stack([res[b]["out"][0, 0:3] for b in range(B)]).astype(np.float32)
    mask_l = np.stack([res[b]["out"][0, 3:6] for b in range(B)]).astype(np.float32)
    label = np.stack([res[b]["out"][0, 6:9] for b in range(B)]).astype(np.float32)
    return (heat, label, mask_l)
